# revision 35
# baseline (speedup 1.0000x reference)
"""CoAttention forward on 8 TRN2 NeuronCores — layout-B + fp8 DoubleRow.

Data-parallel over batch B=64 (8 batches/core). U and C run as f16 3-pass
(~22-bit values); W_q/W_v products run as f16 hi*hi plus an fp8-e5m2
DoubleRow pass that adds both cross terms (hi*lo + lo*hi) directly into
the same PSUM accumulation. G_v/G_q assemble their direct term exactly in
f32 PSUM; only the cross terms (t2', S) pay 2-pass hi/lo f16 cost.

Per batch b (Q [512,1024], V [196,1024], D=1024):
  U    = W_b V^T                [D(e), NV]  3-pass f16, stored hi/lo
  C    = tanh(Q U)              [NQ, NV]    3-pass f16, stored f16
  CT   = C^T                    PE f16 transposes
  per d-half (512):
    WvVT = V W_v^T              [NV, d]   f16 + fp8-DR -> psum + hi/lo sbuf
    per q-chunk: WqQT(f16+DR, psum) -> snapshot hi/lo ->
                 G_q^T += C WvVT (2-pass) -> H_q = tanh f16 -> half-dot
    G_v^T = WvVT(psum) += C^T wqqt (2-pass) -> H_v = tanh f16 -> half-dot
  logits f32 via PE col transpose -> softmax -> a bcast -> v_hat/q_hat STT.
"""
import numpy as np
import ml_dtypes

import concourse.bass as bass
import concourse.mybir as mybir
import concourse.tile as tile
from concourse import bacc
from concourse.bass_utils import run_bass_kernel_spmd
from concourse.masks import make_identity

AF = mybir.ActivationFunctionType
ALU = mybir.AluOpType
AX = mybir.AxisListType
F32 = mybir.dt.float32
F16 = mybir.dt.float16
F8 = mybir.dt.float8e5
DR = mybir.MatmulPerfMode.DoubleRow
E5 = ml_dtypes.float8_e5m2

B, NV, NQ, D = 64, 196, 512, 1024
NCORES = 8
NB = B // NCORES
KD = D // 128             # 8 feature k-chunks
MQ = NQ // 128            # 4 q-chunks
NV1 = NV - 128            # 68 rows in second v-chunk
NVP = 208                 # NV padded so fp8 pair-plane stride % 16 == 0
VROWS = (128, NV1)
N_WARM = 40


def build(nb=NB):
    nc = bacc.Bacc(None, target_bir_lowering=False)

    QTh_d = nc.dram_tensor("QTh", [nb, 128, KD, NQ], F16, kind="ExternalInput")
    QTl_d = nc.dram_tensor("QTl", [nb, 128, KD, NQ], F16, kind="ExternalInput")
    VTh_d = nc.dram_tensor("VTh", [nb, 128, KD, NV], F16, kind="ExternalInput")
    VTl_d = nc.dram_tensor("VTl", [nb, 128, KD, NV], F16, kind="ExternalInput")
    WbTh_d = nc.dram_tensor("WbTh", [128, KD, D], F16, kind="ExternalInput")
    WbTl_d = nc.dram_tensor("WbTl", [128, KD, D], F16, kind="ExternalInput")
    WqTh_d = nc.dram_tensor("WqTh", [128, KD, D], F16, kind="ExternalInput")
    WvTh_d = nc.dram_tensor("WvTh", [128, KD, D], F16, kind="ExternalInput")
    WqP_d = nc.dram_tensor("WqP", [128, KD, 2, D], F8, kind="ExternalInput")
    WvP_d = nc.dram_tensor("WvP", [128, KD, 2, D], F8, kind="ExternalInput")
    QP_d = nc.dram_tensor("QP", [nb, 128, KD, 2, NQ], F8, kind="ExternalInput")
    VP_d = nc.dram_tensor("VP", [nb, 128, KD, 2, NV], F8, kind="ExternalInput")
    whv_d = nc.dram_tensor("whv", [1, D], F16, kind="ExternalInput")
    whq_d = nc.dram_tensor("whq", [1, D], F16, kind="ExternalInput")
    OV_d = nc.dram_tensor("OV", [nb, D], F32, kind="ExternalOutput")
    OQ_d = nc.dram_tensor("OQ", [nb, D], F32, kind="ExternalOutput")

    with tile.TileContext(nc) as tc:
        with (
            tc.tile_pool(name="wsb", bufs=1) as wsb,
            tc.tile_pool(name="iop", bufs=2) as iop,
            tc.tile_pool(name="mid", bufs=1) as mid,
            tc.tile_pool(name="sm", bufs=1) as sm,
            tc.tile_pool(name="psp", bufs=1, space="PSUM") as psp,
        ):
            def wtile(name, src, dt=F16, shape=None):
                t = wsb.tile(shape or [128, KD, D], dt, name=name)
                nc.sync.dma_start(out=t, in_=src[:, :, :] if shape is None
                                  else src[:, :, :, :])
                return t

            # small rows first, then wbt + batch-0 inputs (U deps), then rest
            whv_r16 = wsb.tile([1, D], F16)
            nc.sync.dma_start(out=whv_r16, in_=whv_d[:, :])
            whq_r16 = wsb.tile([1, D], F16)
            nc.sync.dma_start(out=whq_r16, in_=whq_d[:, :])
            wbth = wtile("wbth", WbTh_d)

            def load_v(b):
                vth = iop.tile([128, KD, NV], F16, tag="vth", name=f"vth{b}")
                nc.sync.dma_start(out=vth, in_=VTh_d[b])
                vtl = iop.tile([128, KD, NV], F16, tag="vtl", name=f"vtl{b}")
                nc.sync.dma_start(out=vtl, in_=VTl_d[b])
                return vth, vtl

            def load_q(b):
                qth = iop.tile([128, KD, NQ], F16, tag="qth", name=f"qth{b}")
                nc.sync.dma_start(out=qth, in_=QTh_d[b])
                qtl = iop.tile([128, KD, NQ], F16, tag="qtl", name=f"qtl{b}")
                nc.sync.dma_start(out=qtl, in_=QTl_d[b])
                vpair = iop.tile([128, KD, 2, NVP], F8, tag="vpair", name=f"vp{b}")
                nc.sync.dma_start(out=vpair[:, :, :, :NV], in_=VP_d[b])
                qpair = iop.tile([128, KD, 2, NQ], F8, tag="qpair", name=f"qp{b}")
                nc.sync.dma_start(out=qpair, in_=QP_d[b])
                return qth, qtl, qpair, vpair

            def load_inputs(b):
                vth, vtl = load_v(b)
                qth, qtl, qpair, vpair = load_q(b)
                return qth, qtl, vth, vtl, qpair, vpair

            # batch-0 U deps first: wbth, V, wbtl -- then the rest
            vth0, vtl0 = load_v(0)
            wbtl = wtile("wbtl", WbTl_d)
            qth0, qtl0, qpair0, vpair0 = load_q(0)
            inp0 = (qth0, qtl0, vth0, vtl0, qpair0, vpair0)
            wqth = wtile("wqth", WqTh_d)
            wvth = wtile("wvth", WvTh_d)
            wqp = wtile("wqp", WqP_d, F8, [128, KD, 2, D])
            wvp = wtile("wvp", WvP_d, F8, [128, KD, 2, D])

            identh = wsb.tile([128, 128], F16)
            make_identity(nc, identh)
            identf = wsb.tile([128, 128], F32)
            make_identity(nc, identf)
            ones16 = wsb.tile([1, 128], F16)
            nc.vector.memset(ones16, 1.0)

            # PE warm-up while DMAs stream (keeps HAM window hot)
            pwarm = psp.tile([128, 512], F32, tag="pv", bufs=2, name="pwarm")
            for w in range(N_WARM):
                nc.tensor.matmul(pwarm[:, :128], identh, identh, start=True, stop=True)

            # broadcast w_hv / w_hq rows to [128, D] f16
            whv_b = wsb.tile([128, D], F16)
            whq_b = wsb.tile([128, D], F16)
            for h in range(2):
                hs = slice(h * 512, (h + 1) * 512)
                for bt, row in ((whv_b, whv_r16), (whq_b, whq_r16)):
                    pb = psp.tile([128, 512], F32, tag="puc", bufs=3,
                                  name=f"pbw{h}_{0 if bt is whv_b else 1}")
                    nc.tensor.matmul(pb, ones16, row[:, hs], start=True, stop=True)
                    nc.scalar.copy(bt[:, hs], pb)

            for b in range(nb):
                qth, qtl, vth, vtl, qpair, vpair = \
                    inp0 if b == 0 else load_inputs(b)

                # ---- U = W_b V^T  [e, v], 3-pass, hi/lo ----
                u_h = mid.tile([128, KD, NV], F16, tag="u_h")
                u_l = mid.tile([128, KD, NV], F16, tag="u_l")
                for e in range(KD):
                    es = slice(e * 128, (e + 1) * 128)
                    pu = psp.tile([128, 512], F32, tag="puc", bufs=3, name=f"pu{b}_{e}")
                    n = 0
                    for lh, rh in ((wbth, vth), (wbth, vtl), (wbtl, vth)):
                        for k in range(KD):
                            n += 1
                            nc.tensor.matmul(pu[:, :NV], lh[:, k, es], rh[:, k, :],
                                             start=(n == 1), stop=(n == 3 * KD))
                    nc.scalar.copy(u_h[:, e, :], pu[:, :NV])
                    nc.vector.tensor_sub(u_l[:, e, :], pu[:, :NV], u_h[:, e, :])

                # ---- C = tanh(Q U)  [q, v], 3-pass, f16 ----
                c16 = mid.tile([128, MQ, NV], F16, tag="c16")
                for m in range(MQ):
                    ms = slice(m * 128, (m + 1) * 128)
                    pc = psp.tile([128, 512], F32, tag="puc", bufs=3, name=f"pc{b}_{m}")
                    n = 0
                    for e in range(KD):
                        for lh, rh in ((qth, u_h), (qth, u_l), (qtl, u_h)):
                            n += 1
                            nc.tensor.matmul(pc[:, :NV], lh[:, e, ms], rh[:, e, :],
                                             start=(n == 1), stop=(n == 3 * KD))
                    nc.scalar.activation(c16[:, m, :], pc[:, :NV], AF.Tanh)

                # ---- CT = C^T  [v, q] f16 via 128x128 PE transposes ----
                ct16 = mid.tile([128, 2, NQ], F16, tag="ct16")
                for mv in range(2):
                    rows = VROWS[mv]
                    vs = slice(mv * 128, mv * 128 + rows)
                    for mq in range(MQ):
                        pt = psp.tile([128, 128], F16, tag="pcts", bufs=1,
                                      name=f"pt{b}_{mv}_{mq}")
                        nc.tensor.transpose(pt[:rows, :], c16[:, mq, vs], identh)
                        nc.scalar.copy(ct16[:rows, mv, mq * 128:(mq + 1) * 128],
                                       pt[:rows, :])

                # ---- per d-half: WvVT, G_q^T, G_v^T (+half-dots) ----
                wvvt_h = mid.tile([128, 2, D], F16, tag="wvvt_h")
                wvvt_l = mid.tile([128, 2, D], F16, tag="wvvt_l")
                hvc2 = mid.tile([128, 2, 2], F32, tag="hvc2")
                hqc2 = mid.tile([128, MQ, 2], F32, tag="hqc2")
                dscr = sm.tile([128, NQ], F16, tag="scr")
                for h in range(2):
                    hs = slice(h * 512, (h + 1) * 512)
                    wqqt_h = mid.tile([128, MQ, 512], F16, tag="wqqt_h",
                                      name=f"wqh{b}_{h}")
                    wqqt_l = mid.tile([128, MQ, 512], F16, tag="wqqt_l",
                                      name=f"wql{b}_{h}")
                    # (a) WvVT chunks into pv psum (kept open for (c))
                    pv_t = []
                    for mv in range(2):
                        rows = VROWS[mv]
                        vs = slice(mv * 128, mv * 128 + rows)
                        pvt = psp.tile([128, 512], F32, tag="pv", bufs=2,
                                       name=f"pv{b}_{h}_{mv}")
                        for k in range(KD):
                            nc.tensor.matmul(pvt[:rows, :], vth[:, k, vs],
                                             wvth[:, k, hs],
                                             start=(k == 0), stop=False)
                        for k in range(KD):
                            nc.tensor.matmul(pvt[:rows, :], vpair[:, k, :, vs],
                                             wvp[:, k, :, hs],
                                             start=False, stop=False, perf_mode=DR)
                        nc.scalar.copy(wvvt_h[:rows, mv, hs], pvt[:rows, :])
                        nc.vector.tensor_sub(wvvt_l[:rows, mv, hs], pvt[:rows, :],
                                             wvvt_h[:rows, mv, hs])
                        pv_t.append(pvt)
                    # (b) per q-chunk: WqQT -> snapshot -> += S -> H_q
                    for mq in range(MQ):
                        ms = slice(mq * 128, (mq + 1) * 128)
                        pqt = psp.tile([128, 512], F32, tag="pq", bufs=2,
                                       name=f"pq{b}_{h}_{mq}")
                        for k in range(KD):
                            nc.tensor.matmul(pqt, qth[:, k, ms], wqth[:, k, hs],
                                             start=(k == 0), stop=False)
                        for k in range(KD):
                            nc.tensor.matmul(pqt, qpair[:, k, :, ms],
                                             wqp[:, k, :, hs],
                                             start=False, stop=False, perf_mode=DR)
                        # snapshot WqQT (hi/lo) before S accumulates on top
                        nc.scalar.copy(wqqt_h[:, mq, :], pqt)
                        nc.vector.tensor_sub(wqqt_l[:, mq, :], pqt,
                                             wqqt_h[:, mq, :])
                        n = 0
                        for mv in range(2):
                            rows = VROWS[mv]
                            for rh in (wvvt_h, wvvt_l):
                                n += 1
                                nc.tensor.matmul(pqt, ct16[:rows, mv, ms],
                                                 rh[:rows, mv, hs],
                                                 start=False, stop=(n == 4))
                        hqt = mid.tile([128, 512], F16, tag="hq16c", bufs=2,
                                       name=f"hq{b}_{h}_{mq}")
                        nc.scalar.activation(hqt, pqt, AF.Tanh)
                        nc.vector.scalar_tensor_tensor(
                            out=dscr, in0=hqt, scalar=1.0, in1=whq_b[:, hs],
                            op0=ALU.mult, op1=ALU.mult,
                            accum_out=hqc2[:, mq, h:h + 1])
                    # (c) G_v^T: t2' accumulates onto WvVT psum
                    for mv in range(2):
                        rows = VROWS[mv]
                        vs = slice(mv * 128, mv * 128 + rows)
                        n = 0
                        for mq in range(MQ):
                            for rh in (wqqt_h, wqqt_l):
                                n += 1
                                nc.tensor.matmul(pv_t[mv][:rows, :],
                                                 c16[:, mq, vs], rh[:, mq, :],
                                                 start=False, stop=(n == 2 * MQ))
                        hvt = mid.tile([128, 512], F16, tag="hq16c", bufs=2,
                                       name=f"hv{b}_{h}_{mv}")
                        nc.scalar.activation(hvt[:rows, :], pv_t[mv][:rows, :],
                                             AF.Tanh)
                        nc.vector.scalar_tensor_tensor(
                            out=dscr[:rows, :], in0=hvt[:rows, :], scalar=1.0,
                            in1=whv_b[:rows, hs], op0=ALU.mult, op1=ALU.mult,
                            accum_out=hvc2[:rows, mv, h:h + 1])

                # ---- logits: combine half-dots, PE f32 col transpose ----
                hvc = mid.tile([128, 2], F32, tag="hvc")
                hqc = mid.tile([128, MQ], F32, tag="hqc")
                nc.vector.tensor_add(hvc, hvc2[:, :, 0], hvc2[:, :, 1])
                nc.vector.tensor_add(hqc, hqc2[:, :, 0], hqc2[:, :, 1])

                hps_v = psp.tile([128, 512], F32, tag="puc", bufs=3, name=f"hpv{b}")
                for mv in range(2):
                    rows = VROWS[mv]
                    nc.tensor.transpose(hps_v[0:1, mv * 128:mv * 128 + rows],
                                        hvc[:rows, mv:mv + 1], identf[:rows, :rows])
                hps_q = psp.tile([128, 512], F32, tag="puc", bufs=3, name=f"hpq{b}")
                for mq in range(MQ):
                    nc.tensor.transpose(hps_q[0:1, mq * 128:(mq + 1) * 128],
                                        hqc[:, mq:mq + 1], identf)

                # ---- softmax + broadcast ----
                def softmax_bcast(h_ps, n, tagp):
                    negm = sm.tile([1, 1], F32, tag=f"negm{tagp}")
                    nc.vector.reduce_max(negm, h_ps[0:1, :n], axis=AX.X, negate=True)
                    ex16 = sm.tile([1, n], F16, tag=f"ex16{tagp}")
                    ssum = sm.tile([1, 1], F32, tag=f"ssum{tagp}")
                    nc.scalar.activation(ex16, h_ps[0:1, :n], AF.Exp, bias=negm,
                                         accum_out=ssum)
                    rs = sm.tile([1, 1], F32, tag=f"rs{tagp}")
                    nc.vector.reciprocal(rs, ssum)
                    ones_s = sm.tile([1, 128], F16, tag=f"ones_s{tagp}")
                    nc.scalar.mul(ones_s, ones16, rs)
                    ab_ps = psp.tile([128, 512], F32, tag="puc", bufs=3,
                                     name=f"abps{tagp}{b}")
                    nc.tensor.matmul(ab_ps[:, :n], ones_s, ex16, start=True, stop=True)
                    ab = sm.tile([128, n], F16, tag=f"ab{tagp}")
                    nc.scalar.copy(ab, ab_ps[:, :n])
                    return ab

                av_b = softmax_bcast(hps_v, NV, "v")
                aq_b = softmax_bcast(hps_q, NQ, "q")

                # ---- v_hat / q_hat ----
                vhat_sb = sm.tile([128, KD], F32, tag="vhat")
                qhat_sb = sm.tile([128, KD], F32, tag="qhat")
                scr = dscr
                for k in range(KD):
                    nc.vector.scalar_tensor_tensor(
                        out=scr[:, :NV], in0=vth[:, k, :], scalar=1.0,
                        in1=av_b, op0=ALU.mult, op1=ALU.mult,
                        accum_out=vhat_sb[:, k:k + 1])
                for k in range(KD):
                    nc.vector.scalar_tensor_tensor(
                        out=scr, in0=qth[:, k, :], scalar=1.0,
                        in1=aq_b, op0=ALU.mult, op1=ALU.mult,
                        accum_out=qhat_sb[:, k:k + 1])
                nc.sync.dma_start(out=OV_d[b].rearrange("(k p) -> p k", p=128), in_=vhat_sb)
                nc.sync.dma_start(out=OQ_d[b].rearrange("(k p) -> p k", p=128), in_=qhat_sb)

    nc.finalize()
    return nc


_BUILT = {}


def _split(x):
    hi = x.astype(np.float16)
    lo = (x - hi.astype(np.float32)).astype(np.float16)
    return hi, lo


def _plt(x):
    """[*, D(kp), N] -> [*, 128, KD, N] partition-contiguous layout."""
    s = x.shape
    return np.ascontiguousarray(
        x.reshape(*s[:-2], KD, 128, s[-1]).swapaxes(-3, -2))


def _wpair(hi, lo):
    """fp8 e5m2 DR pair planes [128, KD, 2, D]: plane0=lo, plane1=hi."""
    p = np.stack([lo.astype(E5), hi.astype(E5)], axis=-2)   # [D, 2, D]
    return np.ascontiguousarray(_plt(p.reshape(D, 2 * D)).reshape(128, KD, 2, D))


def kernel(V, Q, W_b, W_v, W_q, w_hv, w_hq, _trace=False):
    V = np.asarray(V, dtype=np.float32)
    Q = np.asarray(Q, dtype=np.float32)
    nb = B // NCORES
    QTh, QTl = _split(Q.transpose(0, 2, 1))      # [B, D, NQ] f16
    VTh, VTl = _split(V.transpose(0, 2, 1))      # [B, D, NV] f16
    QTh, QTl, VTh, VTl = _plt(QTh), _plt(QTl), _plt(VTh), _plt(VTl)
    def _e5(x):
        """Fast f16 -> e5m2 cast (same exponent field; RNE on mantissa)."""
        u = x.view(np.uint16).astype(np.uint32)
        v = ((u + 0x7F + ((u >> 8) & 1)) >> 8).astype(np.uint8)
        return v.view(E5)

    QP = np.ascontiguousarray(
        np.stack([_e5(QTh), _e5(QTl)], axis=3))  # [B,128,KD,2,NQ]
    VP = np.ascontiguousarray(
        np.stack([_e5(VTh), _e5(VTl)], axis=3))
    WbTh, WbTl = _split(np.asarray(W_b, dtype=np.float32).T)
    WqTh, WqTl = _split(np.asarray(W_q, dtype=np.float32).T)
    WvTh, WvTl = _split(np.asarray(W_v, dtype=np.float32).T)
    WqP = _wpair(WqTh, WqTl)
    WvP = _wpair(WvTh, WvTl)
    whv = np.ascontiguousarray(np.asarray(w_hv, dtype=np.float32).reshape(1, D).astype(np.float16))
    whq = np.ascontiguousarray(np.asarray(w_hq, dtype=np.float32).reshape(1, D).astype(np.float16))

    if nb not in _BUILT:
        _BUILT[nb] = build(nb)
    nc = _BUILT[nb]

    in_maps = []
    for c in range(NCORES):
        sl = slice(c * nb, (c + 1) * nb)
        in_maps.append({
            "QTh": np.ascontiguousarray(QTh[sl]), "QTl": np.ascontiguousarray(QTl[sl]),
            "VTh": np.ascontiguousarray(VTh[sl]), "VTl": np.ascontiguousarray(VTl[sl]),
            "WbTh": _plt(WbTh), "WbTl": _plt(WbTl),
            "WqTh": _plt(WqTh), "WvTh": _plt(WvTh),
            "WqP": WqP, "WvP": WvP, "whv": whv, "whq": whq,
            "QP": np.ascontiguousarray(QP[sl]),
            "VP": np.ascontiguousarray(VP[sl]),
        })

    out = run_bass_kernel_spmd(nc, in_maps, core_ids=list(range(NCORES)),
                               trace=_trace)
    v_hat = np.concatenate([out.results[c]["OV"] for c in range(NCORES)], axis=0)
    q_hat = np.concatenate([out.results[c]["OQ"] for c in range(NCORES)], axis=0)
    if _trace:
        kernel._last_exec_ns = out.exec_time_ns
        kernel._last_results = out
    return (v_hat, q_hat)


# revision 37
# speedup vs baseline: 1.1385x; 1.1385x over previous
"""CoAttention forward on 8 TRN2 NeuronCores — layout-B + fp8 DoubleRow.

Data-parallel over batch B=64 (8 batches/core). U and C run as f16 3-pass
(~22-bit values); W_q/W_v products run as f16 hi*hi plus an fp8-e5m2
DoubleRow pass that adds both cross terms (hi*lo + lo*hi) directly into
the same PSUM accumulation. G_v/G_q assemble their direct term exactly in
f32 PSUM; only the cross terms (t2', S) pay 2-pass hi/lo f16 cost.

Per batch b (Q [512,1024], V [196,1024], D=1024):
  U    = W_b V^T                [D(e), NV]  3-pass f16, stored hi/lo
  C    = tanh(Q U)              [NQ, NV]    3-pass f16, stored f16
  CT   = C^T                    PE f16 transposes
  per d-half (512):
    WvVT = V W_v^T              [NV, d]   f16 + fp8-DR -> psum + hi/lo sbuf
    per q-chunk: WqQT(f16+DR, psum) -> snapshot hi/lo ->
                 G_q^T += C WvVT (2-pass) -> H_q = tanh f16 -> half-dot
    G_v^T = WvVT(psum) += C^T wqqt (2-pass) -> H_v = tanh f16 -> half-dot
  logits f32 via PE col transpose -> softmax -> a bcast -> v_hat/q_hat STT.
"""
import numpy as np
import ml_dtypes

import concourse.bass as bass
import concourse.mybir as mybir
import concourse.tile as tile
from concourse import bacc
from concourse.bass_utils import run_bass_kernel_spmd
from concourse.masks import make_identity

AF = mybir.ActivationFunctionType
ALU = mybir.AluOpType
AX = mybir.AxisListType
F32 = mybir.dt.float32
F16 = mybir.dt.float16
F8 = mybir.dt.float8e5
DR = mybir.MatmulPerfMode.DoubleRow
E5 = ml_dtypes.float8_e5m2

B, NV, NQ, D = 64, 196, 512, 1024
NCORES = 8
NB = B // NCORES
KD = D // 128             # 8 feature k-chunks
MQ = NQ // 128            # 4 q-chunks
NV1 = NV - 128            # 68 rows in second v-chunk
NVP = 208                 # NV padded so fp8 pair-plane stride % 16 == 0
VROWS = (128, NV1)
N_WARM = 40


def build(nb=NB):
    nc = bacc.Bacc(None, target_bir_lowering=False)

    QTh_d = nc.dram_tensor("QTh", [nb, 128, KD, NQ], F16, kind="ExternalInput")
    QTl_d = nc.dram_tensor("QTl", [nb, 128, KD, NQ], F16, kind="ExternalInput")
    VTh_d = nc.dram_tensor("VTh", [nb, 128, KD, NV], F16, kind="ExternalInput")
    VTl_d = nc.dram_tensor("VTl", [nb, 128, KD, NV], F16, kind="ExternalInput")
    WbTh_d = nc.dram_tensor("WbTh", [128, KD, D], F16, kind="ExternalInput")
    WbTl_d = nc.dram_tensor("WbTl", [128, KD, D], F16, kind="ExternalInput")
    WqTh_d = nc.dram_tensor("WqTh", [128, KD, D], F16, kind="ExternalInput")
    WvTh_d = nc.dram_tensor("WvTh", [128, KD, D], F16, kind="ExternalInput")
    WqP_d = nc.dram_tensor("WqP", [128, KD, 2, D], F8, kind="ExternalInput")
    WvP_d = nc.dram_tensor("WvP", [128, KD, 2, D], F8, kind="ExternalInput")
    QP_d = nc.dram_tensor("QP", [nb, 128, KD, 2, NQ], F8, kind="ExternalInput")
    VP_d = nc.dram_tensor("VP", [nb, 128, KD, 2, NV], F8, kind="ExternalInput")
    whv_d = nc.dram_tensor("whv", [1, D], F16, kind="ExternalInput")
    whq_d = nc.dram_tensor("whq", [1, D], F16, kind="ExternalInput")
    OV_d = nc.dram_tensor("OV", [nb, D], F32, kind="ExternalOutput")
    OQ_d = nc.dram_tensor("OQ", [nb, D], F32, kind="ExternalOutput")

    with tile.TileContext(nc) as tc:
        with (
            tc.tile_pool(name="wsb", bufs=1) as wsb,
            tc.tile_pool(name="iop", bufs=2) as iop,
            tc.tile_pool(name="mid", bufs=1) as mid,
            tc.tile_pool(name="sm", bufs=1) as sm,
            tc.tile_pool(name="psp", bufs=1, space="PSUM") as psp,
        ):
            def wtile(name, src, dt=F16, shape=None):
                t = wsb.tile(shape or [128, KD, D], dt, name=name)
                nc.sync.dma_start(out=t, in_=src[:, :, :] if shape is None
                                  else src[:, :, :, :])
                return t

            # small rows first, then wbt + batch-0 inputs (U deps), then rest
            whv_r16 = wsb.tile([1, D], F16)
            nc.sync.dma_start(out=whv_r16, in_=whv_d[:, :])
            whq_r16 = wsb.tile([1, D], F16)
            nc.sync.dma_start(out=whq_r16, in_=whq_d[:, :])
            wbth = wtile("wbth", WbTh_d)

            def load_v(b):
                vth = iop.tile([128, KD, NV], F16, tag="vth", name=f"vth{b}")
                nc.sync.dma_start(out=vth, in_=VTh_d[b])
                vtl = iop.tile([128, KD, NV], F16, tag="vtl", name=f"vtl{b}")
                nc.sync.dma_start(out=vtl, in_=VTl_d[b])
                return vth, vtl

            def load_q(b):
                qth = iop.tile([128, KD, NQ], F16, tag="qth", name=f"qth{b}")
                nc.sync.dma_start(out=qth, in_=QTh_d[b])
                qtl = iop.tile([128, KD, NQ], F16, tag="qtl", name=f"qtl{b}")
                nc.sync.dma_start(out=qtl, in_=QTl_d[b])
                vpair = iop.tile([128, KD, 2, NVP], F8, tag="vpair", name=f"vp{b}")
                nc.sync.dma_start(out=vpair[:, :, :, :NV], in_=VP_d[b])
                qpair = iop.tile([128, KD, 2, NQ], F8, tag="qpair", name=f"qp{b}")
                nc.sync.dma_start(out=qpair, in_=QP_d[b])
                return qth, qtl, qpair, vpair

            def load_inputs(b):
                vth, vtl = load_v(b)
                qth, qtl, qpair, vpair = load_q(b)
                return qth, qtl, vth, vtl, qpair, vpair

            # batch-0 U deps first: wbth, V, wbtl -- then the rest
            vth0, vtl0 = load_v(0)
            wbtl = wtile("wbtl", WbTl_d)
            qth0, qtl0, qpair0, vpair0 = load_q(0)
            inp0 = (qth0, qtl0, vth0, vtl0, qpair0, vpair0)
            wqth = wtile("wqth", WqTh_d)
            wvth = wtile("wvth", WvTh_d)
            wqp = wsb.tile([128, KD, 2, D], F8, name="wqp")
            nc.sync.dma_start(out=wqp[:, :4], in_=WqP_d[:, :4])
            nc.sync.dma_start(out=wqp[:, 4:], in_=WqP_d[:, 4:])
            wvp = wsb.tile([128, KD, 2, D], F8, name="wvp")
            nc.sync.dma_start(out=wvp[:, :4], in_=WvP_d[:, :4])
            nc.sync.dma_start(out=wvp[:, 4:], in_=WvP_d[:, 4:])

            identh = wsb.tile([128, 128], F16)
            make_identity(nc, identh)
            identf = wsb.tile([128, 128], F32)
            make_identity(nc, identf)
            ones16 = wsb.tile([1, 128], F16)
            nc.vector.memset(ones16, 1.0)

            # PE warm-up while DMAs stream (keeps HAM window hot)
            pwarm = psp.tile([128, 512], F32, tag="pv", bufs=2, name="pwarm")
            for w in range(N_WARM):
                nc.tensor.matmul(pwarm[:, :128], identh, identh, start=True, stop=True)

            # broadcast w_hv / w_hq rows to [128, D] f16
            whv_b = wsb.tile([128, D], F16)
            whq_b = wsb.tile([128, D], F16)
            for h in range(2):
                hs = slice(h * 512, (h + 1) * 512)
                for bt, row in ((whv_b, whv_r16), (whq_b, whq_r16)):
                    pb = psp.tile([128, 512], F32, tag="puc", bufs=3,
                                  name=f"pbw{h}_{0 if bt is whv_b else 1}")
                    nc.tensor.matmul(pb, ones16, row[:, hs], start=True, stop=True)
                    nc.scalar.copy(bt[:, hs], pb)

            for b in range(nb):
                qth, qtl, vth, vtl, qpair, vpair = \
                    inp0 if b == 0 else load_inputs(b)

                # ---- U = W_b V^T  [e, v], 3-pass, hi/lo ----
                u_h = mid.tile([128, KD, NV], F16, tag="u_h")
                u_l = mid.tile([128, KD, NV], F16, tag="u_l")
                for e in range(KD):
                    es = slice(e * 128, (e + 1) * 128)
                    pu = psp.tile([128, 512], F32, tag="puc", bufs=3, name=f"pu{b}_{e}")
                    n = 0
                    for k in range(KD):
                        for lh, rh in ((wbth, vth), (wbth, vtl), (wbtl, vth)):
                            n += 1
                            nc.tensor.matmul(pu[:, :NV], lh[:, k, es], rh[:, k, :],
                                             start=(n == 1), stop=(n == 3 * KD))
                    nc.scalar.copy(u_h[:, e, :], pu[:, :NV])
                    nc.vector.tensor_sub(u_l[:, e, :], pu[:, :NV], u_h[:, e, :])

                # ---- C = tanh(Q U)  [q, v], 3-pass, f16 ----
                c16 = mid.tile([128, MQ, NV], F16, tag="c16")
                for m in range(MQ):
                    ms = slice(m * 128, (m + 1) * 128)
                    pc = psp.tile([128, 512], F32, tag="puc", bufs=3, name=f"pc{b}_{m}")
                    n = 0
                    for e in range(KD):
                        for lh, rh in ((qth, u_h), (qth, u_l), (qtl, u_h)):
                            n += 1
                            nc.tensor.matmul(pc[:, :NV], lh[:, e, ms], rh[:, e, :],
                                             start=(n == 1), stop=(n == 3 * KD))
                    nc.scalar.activation(c16[:, m, :], pc[:, :NV], AF.Tanh)

                # ---- CT = C^T  [v, q] f16 via 128x128 PE transposes ----
                ct16 = mid.tile([128, 2, NQ], F16, tag="ct16")
                for mv in range(2):
                    rows = VROWS[mv]
                    vs = slice(mv * 128, mv * 128 + rows)
                    for mq in range(MQ):
                        pt = psp.tile([128, 128], F16, tag="pcts", bufs=1,
                                      name=f"pt{b}_{mv}_{mq}")
                        nc.tensor.transpose(pt[:rows, :], c16[:, mq, vs], identh)
                        nc.scalar.copy(ct16[:rows, mv, mq * 128:(mq + 1) * 128],
                                       pt[:rows, :])

                # ---- per d-half: WvVT, G_q^T, G_v^T (+half-dots) ----
                wvvt_h = mid.tile([128, 2, D], F16, tag="wvvt_h")
                wvvt_l = mid.tile([128, 2, D], F16, tag="wvvt_l")
                hvc2 = mid.tile([128, 2, 2], F32, tag="hvc2")
                hqc2 = mid.tile([128, MQ, 2], F32, tag="hqc2")
                dscr = sm.tile([128, NQ], F16, tag="scr")
                for h in range(2):
                    hs = slice(h * 512, (h + 1) * 512)
                    wqqt_h = mid.tile([128, MQ, 512], F16, tag="wqqt_h",
                                      name=f"wqh{b}_{h}")
                    wqqt_l = mid.tile([128, MQ, 512], F16, tag="wqqt_l",
                                      name=f"wql{b}_{h}")
                    # (a) WvVT chunks into pv psum (kept open for (c))
                    pv_t = []
                    for mv in range(2):
                        rows = VROWS[mv]
                        vs = slice(mv * 128, mv * 128 + rows)
                        pvt = psp.tile([128, 512], F32, tag="pv", bufs=2,
                                       name=f"pv{b}_{h}_{mv}")
                        for k in range(KD):
                            nc.tensor.matmul(pvt[:rows, :], vth[:, k, vs],
                                             wvth[:, k, hs],
                                             start=(k == 0), stop=False)
                        for k in range(KD):
                            nc.tensor.matmul(pvt[:rows, :], vpair[:, k, :, vs],
                                             wvp[:, k, :, hs],
                                             start=False, stop=False, perf_mode=DR)
                        nc.scalar.copy(wvvt_h[:rows, mv, hs], pvt[:rows, :])
                        nc.vector.tensor_sub(wvvt_l[:rows, mv, hs], pvt[:rows, :],
                                             wvvt_h[:rows, mv, hs])
                        pv_t.append(pvt)
                    # (b) per q-chunk: WqQT -> snapshot -> += S -> H_q
                    for mq in range(MQ):
                        ms = slice(mq * 128, (mq + 1) * 128)
                        pqt = psp.tile([128, 512], F32, tag="pq", bufs=2,
                                       name=f"pq{b}_{h}_{mq}")
                        for k in range(KD):
                            nc.tensor.matmul(pqt, qth[:, k, ms], wqth[:, k, hs],
                                             start=(k == 0), stop=False)
                        for k in range(KD):
                            nc.tensor.matmul(pqt, qpair[:, k, :, ms],
                                             wqp[:, k, :, hs],
                                             start=False, stop=False, perf_mode=DR)
                        # snapshot WqQT (hi/lo) before S accumulates on top
                        nc.scalar.copy(wqqt_h[:, mq, :], pqt)
                        nc.vector.tensor_sub(wqqt_l[:, mq, :], pqt,
                                             wqqt_h[:, mq, :])
                        n = 0
                        for mv in range(2):
                            rows = VROWS[mv]
                            for rh in (wvvt_h, wvvt_l):
                                n += 1
                                nc.tensor.matmul(pqt, ct16[:rows, mv, ms],
                                                 rh[:rows, mv, hs],
                                                 start=False, stop=(n == 4))
                        hqt = mid.tile([128, 512], F16, tag="hq16c", bufs=2,
                                       name=f"hq{b}_{h}_{mq}")
                        nc.scalar.activation(hqt, pqt, AF.Tanh)
                        nc.vector.scalar_tensor_tensor(
                            out=dscr, in0=hqt, scalar=1.0, in1=whq_b[:, hs],
                            op0=ALU.mult, op1=ALU.mult,
                            accum_out=hqc2[:, mq, h:h + 1])
                    # (c) G_v^T: t2' accumulates onto WvVT psum
                    for mv in range(2):
                        rows = VROWS[mv]
                        vs = slice(mv * 128, mv * 128 + rows)
                        n = 0
                        for mq in range(MQ):
                            for rh in (wqqt_h, wqqt_l):
                                n += 1
                                nc.tensor.matmul(pv_t[mv][:rows, :],
                                                 c16[:, mq, vs], rh[:, mq, :],
                                                 start=False, stop=(n == 2 * MQ))
                        hvt = mid.tile([128, 512], F16, tag="hq16c", bufs=2,
                                       name=f"hv{b}_{h}_{mv}")
                        nc.scalar.activation(hvt[:rows, :], pv_t[mv][:rows, :],
                                             AF.Tanh)
                        nc.vector.scalar_tensor_tensor(
                            out=dscr[:rows, :], in0=hvt[:rows, :], scalar=1.0,
                            in1=whv_b[:rows, hs], op0=ALU.mult, op1=ALU.mult,
                            accum_out=hvc2[:rows, mv, h:h + 1])

                # ---- logits: combine half-dots, PE f32 col transpose ----
                hvc = mid.tile([128, 2], F32, tag="hvc")
                hqc = mid.tile([128, MQ], F32, tag="hqc")
                nc.vector.tensor_add(hvc, hvc2[:, :, 0], hvc2[:, :, 1])
                nc.vector.tensor_add(hqc, hqc2[:, :, 0], hqc2[:, :, 1])

                hps_v = psp.tile([128, 512], F32, tag="puc", bufs=3, name=f"hpv{b}")
                for mv in range(2):
                    rows = VROWS[mv]
                    nc.tensor.transpose(hps_v[0:1, mv * 128:mv * 128 + rows],
                                        hvc[:rows, mv:mv + 1], identf[:rows, :rows])
                hps_q = psp.tile([128, 512], F32, tag="puc", bufs=3, name=f"hpq{b}")
                for mq in range(MQ):
                    nc.tensor.transpose(hps_q[0:1, mq * 128:(mq + 1) * 128],
                                        hqc[:, mq:mq + 1], identf)

                # ---- softmax + broadcast ----
                def softmax_bcast(h_ps, n, tagp):
                    negm = sm.tile([1, 1], F32, tag=f"negm{tagp}")
                    nc.vector.reduce_max(negm, h_ps[0:1, :n], axis=AX.X, negate=True)
                    ex16 = sm.tile([1, n], F16, tag=f"ex16{tagp}")
                    ssum = sm.tile([1, 1], F32, tag=f"ssum{tagp}")
                    nc.scalar.activation(ex16, h_ps[0:1, :n], AF.Exp, bias=negm,
                                         accum_out=ssum)
                    rs = sm.tile([1, 1], F32, tag=f"rs{tagp}")
                    nc.vector.reciprocal(rs, ssum)
                    ones_s = sm.tile([1, 128], F16, tag=f"ones_s{tagp}")
                    nc.scalar.mul(ones_s, ones16, rs)
                    ab_ps = psp.tile([128, 512], F32, tag="puc", bufs=3,
                                     name=f"abps{tagp}{b}")
                    nc.tensor.matmul(ab_ps[:, :n], ones_s, ex16, start=True, stop=True)
                    ab = sm.tile([128, n], F16, tag=f"ab{tagp}")
                    nc.scalar.copy(ab, ab_ps[:, :n])
                    return ab

                av_b = softmax_bcast(hps_v, NV, "v")
                aq_b = softmax_bcast(hps_q, NQ, "q")

                # ---- v_hat / q_hat ----
                vhat_sb = sm.tile([128, KD], F32, tag="vhat")
                qhat_sb = sm.tile([128, KD], F32, tag="qhat")
                scr = dscr
                for k in range(KD):
                    nc.vector.scalar_tensor_tensor(
                        out=scr[:, :NV], in0=vth[:, k, :], scalar=1.0,
                        in1=av_b, op0=ALU.mult, op1=ALU.mult,
                        accum_out=vhat_sb[:, k:k + 1])
                for k in range(KD):
                    nc.vector.scalar_tensor_tensor(
                        out=scr, in0=qth[:, k, :], scalar=1.0,
                        in1=aq_b, op0=ALU.mult, op1=ALU.mult,
                        accum_out=qhat_sb[:, k:k + 1])
                nc.sync.dma_start(out=OV_d[b].rearrange("(k p) -> p k", p=128), in_=vhat_sb)
                nc.sync.dma_start(out=OQ_d[b].rearrange("(k p) -> p k", p=128), in_=qhat_sb)

    nc.finalize()
    return nc


_BUILT = {}


def _split(x):
    hi = x.astype(np.float16)
    lo = (x - hi.astype(np.float32)).astype(np.float16)
    return hi, lo


def _plt(x):
    """[*, D(kp), N] -> [*, 128, KD, N] partition-contiguous layout."""
    s = x.shape
    return np.ascontiguousarray(
        x.reshape(*s[:-2], KD, 128, s[-1]).swapaxes(-3, -2))


def _wpair(hi, lo):
    """fp8 e5m2 DR pair planes [128, KD, 2, D]: plane0=lo, plane1=hi."""
    p = np.stack([lo.astype(E5), hi.astype(E5)], axis=-2)   # [D, 2, D]
    return np.ascontiguousarray(_plt(p.reshape(D, 2 * D)).reshape(128, KD, 2, D))


def kernel(V, Q, W_b, W_v, W_q, w_hv, w_hq, _trace=False):
    V = np.asarray(V, dtype=np.float32)
    Q = np.asarray(Q, dtype=np.float32)
    nb = B // NCORES
    QTh, QTl = _split(Q.transpose(0, 2, 1))      # [B, D, NQ] f16
    VTh, VTl = _split(V.transpose(0, 2, 1))      # [B, D, NV] f16
    QTh, QTl, VTh, VTl = _plt(QTh), _plt(QTl), _plt(VTh), _plt(VTl)
    def _e5(x):
        """Fast f16 -> e5m2 cast (same exponent field; RNE on mantissa)."""
        u = x.view(np.uint16).astype(np.uint32)
        v = ((u + 0x7F + ((u >> 8) & 1)) >> 8).astype(np.uint8)
        return v.view(E5)

    QP = np.ascontiguousarray(
        np.stack([_e5(QTh), _e5(QTl)], axis=3))  # [B,128,KD,2,NQ]
    VP = np.ascontiguousarray(
        np.stack([_e5(VTh), _e5(VTl)], axis=3))
    WbTh, WbTl = _split(np.asarray(W_b, dtype=np.float32).T)
    WqTh, WqTl = _split(np.asarray(W_q, dtype=np.float32).T)
    WvTh, WvTl = _split(np.asarray(W_v, dtype=np.float32).T)
    WqP = _wpair(WqTh, WqTl)
    WvP = _wpair(WvTh, WvTl)
    whv = np.ascontiguousarray(np.asarray(w_hv, dtype=np.float32).reshape(1, D).astype(np.float16))
    whq = np.ascontiguousarray(np.asarray(w_hq, dtype=np.float32).reshape(1, D).astype(np.float16))

    if nb not in _BUILT:
        _BUILT[nb] = build(nb)
    nc = _BUILT[nb]

    in_maps = []
    for c in range(NCORES):
        sl = slice(c * nb, (c + 1) * nb)
        in_maps.append({
            "QTh": np.ascontiguousarray(QTh[sl]), "QTl": np.ascontiguousarray(QTl[sl]),
            "VTh": np.ascontiguousarray(VTh[sl]), "VTl": np.ascontiguousarray(VTl[sl]),
            "WbTh": _plt(WbTh), "WbTl": _plt(WbTl),
            "WqTh": _plt(WqTh), "WvTh": _plt(WvTh),
            "WqP": WqP, "WvP": WvP, "whv": whv, "whq": whq,
            "QP": np.ascontiguousarray(QP[sl]),
            "VP": np.ascontiguousarray(VP[sl]),
        })

    out = run_bass_kernel_spmd(nc, in_maps, core_ids=list(range(NCORES)),
                               trace=_trace)
    v_hat = np.concatenate([out.results[c]["OV"] for c in range(NCORES)], axis=0)
    q_hat = np.concatenate([out.results[c]["OQ"] for c in range(NCORES)], axis=0)
    if _trace:
        kernel._last_exec_ns = out.exec_time_ns
        kernel._last_results = out
    return (v_hat, q_hat)
